# revision 55
# baseline (speedup 1.0000x reference)
"""EulerAttention Trainium2 kernel (v4: fused single-sweep pipeline).

Per-core sharding: core c in 0..7 -> (batch b = c // 4, query block qb = c % 4,
1024 queries each).  Each core computes K/V for its whole batch, Q features for
its query block, then flash-style scores/softmax/AV.

Key ideas:
- Scores/rowsum/AV are invariant under key permutation, so the host hands each
  core its batch x with the key blocks rotated to put the core's own query rows
  first.  Q features then reuse the same x loads as key blocks 0-1 and the
  whole kernel runs as ONE software-pipelined loop over key blocks:
  K features (ACT/DVE/Pool-heavy) overlap V projection + fp8 scores + AV
  (PE-heavy) of neighbouring blocks.
- All projections f32r for e-tiles 0..FP8_FROM-1 (phase-precision-critical,
  the 1/wavelength scale amplifies projection error), fp8 DoubleRow for the
  rest.  Q/K features (cos/sin theta) are stored fp8 and the [S,S] score
  matmuls run fp8 DoubleRow (2 feature tiles per pass, 4x f32r throughput).
- K-side features are mean-centered per feature (host-computed E[cos theta_k]
  from the weights); the dropped cross terms are per-query constants that
  cancel in softmax normalization.  Centering shrinks fp8 quantization noise
  of the near-constant long-wavelength features.
- V, exp(scores), and the output accumulator are bf16; V and K features stay
  SBUF-resident in rotating per-block slots (no DRAM roundtrips).
- Feature-map range reduction (turns-space magic round + add_range_wrap into
  the ACT Sin table) is spread across DVE and Pool (gpsimd).

kernel(**inputs) takes the full unsharded inputs from reference.setup_inputs()
and returns the full [B, S, D] output.
"""
import sys, math

sys.path.insert(0, "/opt/trn_rl_repo")

import numpy as np
import ml_dtypes

B, S, D = 2, 4096, 1024
NCORES = 8
QBLK = S // 4          # queries per core
ET = D // 128          # number of 128-row e/d tiles (8)
NSLOT = 2 * ET         # fp8 feature slots (cos/sin per et)
MAGIC = float(1.5 * 2**23)
TWOPI = 2.0 * math.pi
INV_SQRT_D = 1.0 / math.sqrt(D)
F8NP = ml_dtypes.float8_e4m3
FP8_FROM = 2           # e-tiles >= this run the Q/K projection in fp8 DoubleRow

_cache = {}


def _build_program(trace_sim=False):
    import concourse.bass as bass
    from concourse import bacc
    import concourse.mybir as mybir
    import concourse.tile as tile
    from contextlib import ExitStack

    f32 = mybir.dt.float32
    f32r = mybir.dt.float32r
    bf16 = mybir.dt.bfloat16
    f8 = mybir.dt.float8e4
    Act = mybir.ActivationFunctionType
    Alu = mybir.AluOpType
    PM = mybir.MatmulPerfMode

    s_keys, s_q = S, QBLK
    n_sblk = s_keys // 512       # 8 key blocks (block 0-1 = this core's queries)
    n_tt = s_keys // 128         # 32 key tiles
    n_qsb = s_q // 512           # 2 query blocks
    NS = s_q                     # resident query width (1024)
    n_ns = NS // 512             # N-splits for matmuls over queries

    nc = bacc.Bacc("TRN2", target_bir_lowering=False, debug=False)

    xT = nc.dram_tensor("xT", [D, s_keys], f32r, kind="ExternalInput").ap()
    XT8 = nc.dram_tensor("XT8", [D, s_keys], f8, kind="ExternalInput").ap()
    # f32r weight slices for the low e-tiles; fp8 full weights for DoubleRow
    WqT = nc.dram_tensor("WqT", [D, FP8_FROM * 128], f32r, kind="ExternalInput").ap()
    WkT = nc.dram_tensor("WkT", [D, FP8_FROM * 128], f32r, kind="ExternalInput").ap()
    WQ8 = nc.dram_tensor("WQ8", [D, D], f8, kind="ExternalInput").ap()
    WK8 = nc.dram_tensor("WK8", [D, D], f8, kind="ExternalInput").ap()
    WvT = nc.dram_tensor("WvT", [D, D], f32r, kind="ExternalInput").ap()
    # packed per-partition constants: columns = (sc2 | bq2 | bk2 | bv | nac | nas) x ET
    CON = nc.dram_tensor("CON", [128, 6 * ET], f32, kind="ExternalInput").ap()

    OT = nc.dram_tensor("OT", [D, s_q], f32, kind="ExternalOutput").ap()

    with tile.TileContext(nc, trace_sim=trace_sim) as tc, ExitStack() as top:
        # ---- constants ----
        cpool = top.enter_context(tc.tile_pool(name="consts", bufs=1))
        ctile = cpool.tile([128, 6 * ET], f32, tag="ctile")
        nc.sync.dma_start(ctile[:], CON[:])
        sc2 = [ctile[:, i : i + 1] for i in range(ET)]
        bq2 = [ctile[:, ET + i : ET + i + 1] for i in range(ET)]
        bk2 = [ctile[:, 2 * ET + i : 2 * ET + i + 1] for i in range(ET)]
        bvt = [ctile[:, 3 * ET + i : 3 * ET + i + 1] for i in range(ET)]
        nac = [ctile[:, 4 * ET + i : 4 * ET + i + 1] for i in range(ET)]
        nas = [ctile[:, 5 * ET + i : 5 * ET + i + 1] for i in range(ET)]
        ones_col = cpool.tile([128, 2, 128], f8, tag="ones_col")  # DR rowsum lhsT
        nc.vector.memset(ones_col[:], 1.0)
        ones_row = cpool.tile([1, 128], bf16, tag="ones_row")
        nc.vector.memset(ones_row[:], 1.0)

        psum = top.enter_context(tc.tile_pool(name="psum", bufs=1, space="PSUM"))

        # ---- output accumulator (bf16) ----
        oacc = top.enter_context(tc.tile_pool(name="oacc", bufs=1))
        o_ac = [oacc.tile([128, NS], bf16, tag=f"o{dt}", name=f"oac{dt}")
                for dt in range(ET)]

        # ---- resident Q features + rotating K-feature / V slots ----
        qres = top.enter_context(tc.tile_pool(name="qres", bufs=1))
        qa8 = qres.tile([128, NSLOT, NS], f8, tag="qa8")
        kpool = top.enter_context(tc.tile_pool(name="kres", bufs=2))
        vpool = top.enter_context(tc.tile_pool(name="vres", bufs=1))
        vres = [vpool.tile([128, 4, D], bf16, tag=f"v{i}", name=f"v{i}")
                for i in range(2)]

        p1 = top.enter_context(tc.tile_pool(name="p1sb", bufs=2))
        w1 = top.enter_context(tc.tile_pool(name="w1", bufs=1))
        pch = top.enter_context(tc.tile_pool(name="pch", bufs=2))
        epool = top.enter_context(tc.tile_pool(name="epool", bufs=8))
        pnorm = top.enter_context(tc.tile_pool(name="pnorm", bufs=1))

        wq = [w1.tile([128, FP8_FROM * 128], f32r, tag=f"wq{d}", name=f"wq{d}")
              for d in range(ET)]
        wk = [w1.tile([128, FP8_FROM * 128], f32r, tag=f"wk{d}", name=f"wk{d}")
              for d in range(ET)]
        wq8 = [w1.tile([128, 2, D], f8, tag=f"wq8{m}", name=f"wq8{m}")
               for m in range(ET // 2)]
        wk8 = [w1.tile([128, 2, D], f8, tag=f"wk8{m}", name=f"wk8{m}")
               for m in range(ET // 2)]
        wv = [w1.tile([128, D], f32r, tag=f"wv{d}", name=f"wv{d}")
              for d in range(ET)]

        def qslot(et, cs, qsb):
            return qa8[:, 2 * et + cs, qsb * 512 : qsb * 512 + 512]

        def load_xblk(col0, between=None):
            br = p1.tile([128, ET * 512], f32r, tag="xbr", name="xbr")
            # two half-loads so the first projection chain starts sooner
            for h in range(2):
                nc.sync.dma_start(
                    br[:, h * 4 * 512 : (h + 1) * 4 * 512]
                    .rearrange("p (d s) -> p d s", d=4),
                    xT[h * 4 * 128 : (h + 1) * 4 * 128, col0 : col0 + 512]
                    .rearrange("(d p) s -> p d s", p=128))
                if h == 0 and between is not None:
                    between()
            x8 = p1.tile([128, ET, 512], f8, tag="x8", name="x8")
            nc.sync.dma_start(
                x8[:],
                XT8[:, col0 : col0 + 512].rearrange("(d p) s -> p d s", p=128))
            return br, x8

        def theta_chain(xb, et, w_tiles, w8_tiles, bias_tiles, r_on_dve=False):
            """Projection + range reduction; returns (f, g) turn tiles for Sin."""
            br, x8 = xb
            ps = psum.tile([128, 512], f32, tag="proj", name="psf", bufs=4)
            if et < FP8_FROM:
                for d in range(ET):
                    nc.tensor.matmul(ps[:], w_tiles[d][:, et * 128 : (et + 1) * 128],
                                     br[:, d * 512 : (d + 1) * 512],
                                     start=(d == 0), stop=(d == ET - 1))
            else:
                for m in range(ET // 2):
                    nc.tensor.matmul(ps[:],
                                     w8_tiles[m][:, :, et * 128 : (et + 1) * 128],
                                     x8[:, 2 * m : 2 * m + 2, :],
                                     start=(m == 0), stop=(m == ET // 2 - 1),
                                     perf_mode=PM.DoubleRow)
            r = pch.tile([128, 512], f32, tag="r", name="r")
            if r_on_dve:
                nc.vector.tensor_scalar(r[:], ps[:], sc2[et][:], bias_tiles[et][:],
                                        Alu.mult, Alu.add)
            else:
                nc.scalar.activation(r[:], ps[:], Act.Identity,
                                     scale=sc2[et][:], bias=bias_tiles[et][:])
            kk = pch.tile([128, 512], f32, tag="kk", name="kk")
            nc.gpsimd.tensor_scalar(kk[:], r[:], MAGIC, MAGIC, Alu.add, Alu.subtract)
            f = pch.tile([128, 512], f32, tag="f", name="f")
            nc.vector.scalar_tensor_tensor(f[:], kk[:], -1.0, r[:],
                                           Alu.mult, Alu.add)
            g = pch.tile([128, 512], f32, tag="kk", name="g")
            nc.vector.add_range_wrap(g[:], f[:], 0.25, 0.5, 1.0)
            return f, g

        ps_rs = psum.tile([128, NS], f32, tag="rs", bufs=1)
        e_group = []
        e8_pairs = []
        kq = []  # deferred (sblk, kres_tile) for score emission

        def kfeature_unit(xb, kr, et):
            f, g = theta_chain(xb, et, wk, wk8, bk2)
            s32 = pch.tile([128, 512], f32, tag="s32", name="s32")
            nc.scalar.activation(s32[:], f[:], Act.Sin, scale=TWOPI)
            nc.gpsimd.tensor_scalar(kr[:, 2 * et + 1, :], s32[:],
                                    nas[et][:], None, Alu.add)
            c32 = pch.tile([128, 512], f32, tag="s32", name="c32")
            nc.scalar.activation(c32[:], g[:], Act.Sin, scale=TWOPI)
            nc.gpsimd.tensor_scalar(kr[:, 2 * et, :], c32[:],
                                    nac[et][:], None, Alu.add)

        def qfeature_unit(xb, qsb, et):
            f, g = theta_chain(xb, et, wq, wq8, bq2, r_on_dve=True)
            nc.scalar.activation(qslot(et, 1, qsb), f[:], Act.Sin, scale=TWOPI)
            nc.scalar.activation(qslot(et, 0, qsb), g[:], Act.Sin, scale=TWOPI)

        def vproj_unit(xb, sblk, ti):
            br = xb[0]
            for dg in range(2):
                psv = psum.tile([128, 512], f32, tag="proj", name="psv",
                                bufs=4)
                for d in range(ET):
                    nc.tensor.matmul(
                        psv[:],
                        br[:, d * 512 + ti * 128 : d * 512 + (ti + 1) * 128],
                        wv[d][:, dg * 512 : dg * 512 + 512],
                        start=(d == 0), stop=(d == ET - 1))
                nc.vector.tensor_copy(
                    vres[sblk % 2][:, ti, dg * 512 : (dg + 1) * 512], psv[:])

        def emit_block_units(xb, kr, sblk):
            """K features (+ Q features for the query blocks) with the V
            projection interleaved so the PE never starves while ACT/DVE/Pool
            chew the feature chains."""
            units = []
            for et in range(ET):
                units.append(lambda et=et: kfeature_unit(xb, kr, et))
                if sblk < n_qsb:
                    units.append(lambda et=et: qfeature_unit(xb, sblk, et))
            vus = [lambda ti=ti: vproj_unit(xb, sblk, ti) for ti in range(4)]
            # pattern: two feature units up front, then alternate
            order = []
            vi = 0
            for i, u in enumerate(units):
                order.append(u)
                if i >= 1 and (i % (len(units) // 4)) == 1 and vi < 4:
                    order.append(vus[vi]); vi += 1
            order.extend(vus[vi:])
            for u in order:
                u()

        def emit_scores(kr, sblk):
            for loc in range(4):
                tt = sblk * 4 + loc
                et_t = epool.tile([128, NS], bf16, tag="e", name="e")
                for ns in range(n_ns):
                    sl = slice(ns * 512, ns * 512 + 512)
                    ps_sim = psum.tile([128, 512], f32, tag="big",
                                       name="ps_sim", bufs=2)
                    for j in range(ET):
                        nc.tensor.matmul(
                            ps_sim[:],
                            kr[:, 2 * j : 2 * j + 2, loc * 128 : (loc + 1) * 128],
                            qa8[:, 2 * j : 2 * j + 2, ns * 512 : ns * 512 + 512],
                            start=(j == 0), stop=(j == ET - 1),
                            perf_mode=PM.DoubleRow)
                    nc.scalar.activation(et_t[:, sl], ps_sim[:], Act.Exp,
                                         scale=INV_SQRT_D)
                e_group.append((tt, et_t))
                if len(e_group) % 2 == 0:
                    # fp8 pair copy for the DoubleRow rowsum (reuses x8 slots)
                    e8 = p1.tile([128, 2, NS], f8, tag="x8", name="e8")
                    nc.gpsimd.tensor_copy(e8[:, 0, :], e_group[-2][1][:])
                    nc.gpsimd.tensor_copy(e8[:, 1, :], e_group[-1][1][:])
                    e8_pairs.append((e_group[-2][0], e8))

        def emit_av(tg):
            # rowsums first (exps are long done; avoids pacing PE on ACT);
            # fp8 DoubleRow: 2 key-tiles per pass
            for ptt, e8 in e8_pairs:
                for ns in range(n_ns):
                    sl = slice(ns * 512, ns * 512 + 512)
                    nc.tensor.matmul(ps_rs[:, sl], ones_col[:], e8[:, :, sl],
                                     start=(ptt == 0), stop=(ptt == n_tt - 2),
                                     perf_mode=PM.DoubleRow)
            e8_pairs.clear()
            if tg == n_sblk // 2 - 1:
                # rowsum chain closed: 1/rowsum + broadcast overlap final AV
                rec = pnorm.tile([1, NS], bf16, tag="rec")
                with nc.allow_low_precision(
                        reason="1/rowsum broadcast runs bf16; it feeds the "
                               "bf16 bc tile"):
                    nc.vector.reciprocal(rec[:], ps_rs[:1, :])
                bc_t = pnorm.tile([128, NS], bf16, tag="bc")
                for ns in range(n_ns):
                    sl = slice(ns * 512, ns * 512 + 512)
                    ps_bc = psum.tile([128, 512], f32, tag="big", name="ps_bc",
                                      bufs=2)
                    nc.tensor.matmul(ps_bc[:], ones_row[:], rec[:, sl],
                                     start=True, stop=True)
                    nc.vector.tensor_copy(bc_t[:, sl], ps_bc[:])
                bc.append(bc_t)
            for dg in range(2):
                for di in range(4):
                    dt = dg * 4 + di
                    for ns in range(n_ns):
                        sl = slice(ns * 512, ns * 512 + 512)
                        ps_o = psum.tile([128, 512], f32, tag="big", name="ps_o",
                                         bufs=2)
                        for gi in range(8):
                            g_s, ti = gi // 4, gi % 4
                            nc.tensor.matmul(
                                ps_o[:],
                                vres[g_s][:, ti, dt * 128 : (dt + 1) * 128],
                                e_group[gi][1][:, sl],
                                start=(gi == 0), stop=(gi == 7))
                        if tg == 0:
                            nc.vector.tensor_copy(o_ac[dt][:, sl], ps_o[:])
                        else:
                            nc.vector.tensor_tensor(o_ac[dt][:, sl], ps_o[:],
                                                    o_ac[dt][:, sl], Alu.add)
                    if tg == n_sblk // 2 - 1:
                        # final group: normalize + V bias + store right away so
                        # the tail pipelines behind the remaining AV chains
                        on = p1.tile([128, NS], f32, tag="xbr", name="on")
                        nc.vector.tensor_tensor(on[:], o_ac[dt][:], bc[0][:],
                                                Alu.mult)
                        nc.scalar.activation(on[:], on[:], Act.Identity,
                                             bias=bvt[dt][:])
                        nc.sync.dma_start(OT[dt * 128 : (dt + 1) * 128, :], on[:])

        def emit_scores_and_maybe_av(s_i, kr_i):
            emit_scores(kr_i, s_i)
            if len(e_group) == 8:
                emit_av(s_i // 2)
                e_group.clear()

        # ---- main pipeline over key blocks ----
        bc = []
        def load_wk():
            for d in range(ET):
                nc.sync.dma_start(wk[d][:], WkT[d * 128 : (d + 1) * 128, :])

        for sblk in range(n_sblk):
            xb = load_xblk(sblk * 512, between=load_wk if sblk == 0 else None)
            if sblk == 0:
                # remaining weights behind the first x block on the DMA queue
                for d in range(ET):
                    nc.sync.dma_start(wq[d][:], WqT[d * 128 : (d + 1) * 128, :])
                for m in range(ET // 2):
                    nc.sync.dma_start(
                        wk8[m][:], WK8[2 * m * 128 : (2 * m + 2) * 128, :]
                        .rearrange("(j p) e -> p j e", p=128))
                for m in range(ET // 2):
                    nc.sync.dma_start(
                        wq8[m][:], WQ8[2 * m * 128 : (2 * m + 2) * 128, :]
                        .rearrange("(j p) e -> p j e", p=128))
                for d in range(ET):
                    nc.sync.dma_start(wv[d][:], WvT[d * 128 : (d + 1) * 128, :])
            kr = kpool.tile([128, NSLOT, 512], f8, tag="kr", name=f"kr{sblk}")
            kq.append((sblk, kr))
            if sblk == n_qsb:
                # qa8 complete: flush scores (+AV) of the query blocks BEFORE
                # vproj(sblk) (inside emit_block_units) reuses their vres slots
                while kq[0][0] < n_qsb:
                    s_i, kr_i = kq.pop(0)
                    emit_scores_and_maybe_av(s_i, kr_i)
            emit_block_units(xb, kr, sblk)
            if sblk >= n_qsb:
                s_i, kr_i = kq.pop(0)
                emit_scores_and_maybe_av(s_i, kr_i)

    nc.compile()
    return nc


def _host_prep(x, Wq, bq, Wk, bk, Wv, bv, phase_bias):
    wavelengths = np.arange(1, D + 1, dtype=np.float32) * np.float32(2.0 * math.pi / D)
    inv_wl = (np.float32(1.0) / (wavelengths + np.float32(1e-8))).astype(np.float32)
    sc2 = (inv_wl / TWOPI).astype(np.float32).reshape(ET, 128)
    bq2 = ((bq * inv_wl + phase_bias) / TWOPI).astype(np.float32).reshape(ET, 128)
    bk2 = ((bk * inv_wl + phase_bias) / TWOPI).astype(np.float32).reshape(ET, 128)
    # K-feature means from the weights: theta_k ~ N(bk*ivl + pb, |wk_row|^2 ivl^2)
    mu = (bk * inv_wl + phase_bias).astype(np.float64)
    var = (np.sum(Wk.astype(np.float64) ** 2, axis=1) * inv_wl.astype(np.float64) ** 2)
    damp = np.exp(-var / 2.0)
    nac = (-(np.cos(mu) * damp)).astype(np.float32).reshape(ET, 128)
    nas = (-(np.sin(mu) * damp)).astype(np.float32).reshape(ET, 128)
    WqTf = np.ascontiguousarray(Wq.T).astype(np.float32)
    WkTf = np.ascontiguousarray(Wk.T).astype(np.float32)
    WqT = np.ascontiguousarray(WqTf[:, : FP8_FROM * 128])
    WkT = np.ascontiguousarray(WkTf[:, : FP8_FROM * 128])
    WQ8 = WqTf.astype(F8NP)
    WK8 = WkTf.astype(F8NP)
    WvT = np.ascontiguousarray(Wv.T).astype(np.float32)
    xT = [np.ascontiguousarray(x[b].T).astype(np.float32) for b in range(x.shape[0])]
    con = np.stack([sc2, bq2, bk2, bv.reshape(ET, 128).astype(np.float32), nac, nas])
    # [6, ET, 128] -> [128, 6*ET] with column layout (kind, et)
    con = np.ascontiguousarray(con.reshape(6 * ET, 128).T).astype(np.float32)
    return xT, WqT, WkT, WQ8, WK8, WvT, con


def kernel(x, Wq, bq, Wk, bk, Wv, bv, phase_bias, _trace=False):
    from concourse.bass_utils import run_bass_kernel_spmd

    x = np.asarray(x, dtype=np.float32)
    xT, WqT, WkT, WQ8, WK8, WvT, con = _host_prep(
        x, np.asarray(Wq, np.float32), np.asarray(bq, np.float32),
        np.asarray(Wk, np.float32), np.asarray(bk, np.float32),
        np.asarray(Wv, np.float32), np.asarray(bv, np.float32),
        np.asarray(phase_bias, np.float32))

    if "prog" not in _cache:
        _cache["prog"] = _build_program()
    nc = _cache["prog"]

    in_maps = []
    for c in range(NCORES):
        b, qb = c // 4, c % 4
        # rotate the key blocks so this core's query rows come first
        # (scores/rowsum/AV are invariant under key permutation)
        xp = np.concatenate(
            [xT[b][:, qb * QBLK : (qb + 1) * QBLK],
             xT[b][:, : qb * QBLK],
             xT[b][:, (qb + 1) * QBLK :]], axis=1)
        xp = np.ascontiguousarray(xp)
        in_maps.append({
            "xT": xp,
            "XT8": xp.astype(F8NP),
            "WqT": WqT, "WkT": WkT, "WQ8": WQ8, "WK8": WK8, "WvT": WvT,
            "CON": con,
        })
    res = run_bass_kernel_spmd(nc, in_maps, core_ids=list(range(NCORES)),
                               trace=_trace)
    out = np.empty((B, S, D), dtype=np.float32)
    for c in range(NCORES):
        b, qb = c // 4, c % 4
        out[b, qb * QBLK : (qb + 1) * QBLK, :] = res.results[c]["OT"].T
    if _trace:
        kernel.last_exec_time_ns = res.exec_time_ns
        kernel.last_result = res
    return out


# revision 59
# speedup vs baseline: 1.0065x; 1.0065x over previous
"""EulerAttention Trainium2 kernel (v4: fused single-sweep pipeline).

Per-core sharding: core c in 0..7 -> (batch b = c // 4, query block qb = c % 4,
1024 queries each).  Each core computes K/V for its whole batch, Q features for
its query block, then flash-style scores/softmax/AV.

Key ideas:
- Scores/rowsum/AV are invariant under key permutation, so the host hands each
  core its batch x with the key blocks rotated to put the core's own query rows
  first.  Q features then reuse the same x loads as key blocks 0-1 and the
  whole kernel runs as ONE software-pipelined loop over key blocks:
  K features (ACT/DVE/Pool-heavy) overlap V projection + fp8 scores + AV
  (PE-heavy) of neighbouring blocks.
- All projections f32r for e-tiles 0..FP8_FROM-1 (phase-precision-critical,
  the 1/wavelength scale amplifies projection error), fp8 DoubleRow for the
  rest.  Q/K features (cos/sin theta) are stored fp8 and the [S,S] score
  matmuls run fp8 DoubleRow (2 feature tiles per pass, 4x f32r throughput).
- K-side features are mean-centered per feature (host-computed E[cos theta_k]
  from the weights); the dropped cross terms are per-query constants that
  cancel in softmax normalization.  Centering shrinks fp8 quantization noise
  of the near-constant long-wavelength features.
- V, exp(scores), and the output accumulator are bf16; V and K features stay
  SBUF-resident in rotating per-block slots (no DRAM roundtrips).
- Feature-map range reduction (turns-space magic round + add_range_wrap into
  the ACT Sin table) is spread across DVE and Pool (gpsimd).

kernel(**inputs) takes the full unsharded inputs from reference.setup_inputs()
and returns the full [B, S, D] output.
"""
import sys, math

sys.path.insert(0, "/opt/trn_rl_repo")

import numpy as np
import ml_dtypes

B, S, D = 2, 4096, 1024
NCORES = 8
QBLK = S // 4          # queries per core
ET = D // 128          # number of 128-row e/d tiles (8)
NSLOT = 2 * ET         # fp8 feature slots (cos/sin per et)
MAGIC = float(1.5 * 2**23)
TWOPI = 2.0 * math.pi
INV_SQRT_D = 1.0 / math.sqrt(D)
F8NP = ml_dtypes.float8_e4m3
FP8_FROM = 2           # e-tiles >= this run the Q/K projection in fp8 DoubleRow

_cache = {}


def _build_program(trace_sim=False):
    import concourse.bass as bass
    from concourse import bacc
    import concourse.mybir as mybir
    import concourse.tile as tile
    from contextlib import ExitStack

    f32 = mybir.dt.float32
    f32r = mybir.dt.float32r
    bf16 = mybir.dt.bfloat16
    f8 = mybir.dt.float8e4
    Act = mybir.ActivationFunctionType
    Alu = mybir.AluOpType
    PM = mybir.MatmulPerfMode

    s_keys, s_q = S, QBLK
    n_sblk = s_keys // 512       # 8 key blocks (block 0-1 = this core's queries)
    n_tt = s_keys // 128         # 32 key tiles
    n_qsb = s_q // 512           # 2 query blocks
    NS = s_q                     # resident query width (1024)
    n_ns = NS // 512             # N-splits for matmuls over queries

    nc = bacc.Bacc("TRN2", target_bir_lowering=False, debug=False)

    xT = nc.dram_tensor("xT", [D, s_keys], f32r, kind="ExternalInput").ap()
    XT8 = nc.dram_tensor("XT8", [D, s_keys], f8, kind="ExternalInput").ap()
    # f32r weight slices for the low e-tiles; fp8 full weights for DoubleRow
    WqT = nc.dram_tensor("WqT", [D, FP8_FROM * 128], f32r, kind="ExternalInput").ap()
    WkT = nc.dram_tensor("WkT", [D, FP8_FROM * 128], f32r, kind="ExternalInput").ap()
    WQ8 = nc.dram_tensor("WQ8", [D, D], f8, kind="ExternalInput").ap()
    WK8 = nc.dram_tensor("WK8", [D, D], f8, kind="ExternalInput").ap()
    WvT = nc.dram_tensor("WvT", [D, D], f32r, kind="ExternalInput").ap()
    # packed per-partition constants: columns = (sc2 | bq2 | bk2 | bv | nac | nas) x ET
    CON = nc.dram_tensor("CON", [128, 6 * ET], f32, kind="ExternalInput").ap()

    OT = nc.dram_tensor("OT", [D, s_q], f32, kind="ExternalOutput").ap()

    with tile.TileContext(nc, trace_sim=trace_sim) as tc, ExitStack() as top:
        # ---- constants ----
        cpool = top.enter_context(tc.tile_pool(name="consts", bufs=1))
        ctile = cpool.tile([128, 6 * ET], f32, tag="ctile")
        nc.sync.dma_start(ctile[:], CON[:])
        sc2 = [ctile[:, i : i + 1] for i in range(ET)]
        bq2 = [ctile[:, ET + i : ET + i + 1] for i in range(ET)]
        bk2 = [ctile[:, 2 * ET + i : 2 * ET + i + 1] for i in range(ET)]
        bvt = [ctile[:, 3 * ET + i : 3 * ET + i + 1] for i in range(ET)]
        nac = [ctile[:, 4 * ET + i : 4 * ET + i + 1] for i in range(ET)]
        nas = [ctile[:, 5 * ET + i : 5 * ET + i + 1] for i in range(ET)]
        ones_col = cpool.tile([128, 2], bf16, tag="ones_col")
        nc.vector.memset(ones_col[:], 1.0)
        ones_row = cpool.tile([1, 128], bf16, tag="ones_row")
        nc.vector.memset(ones_row[:], 1.0)

        psum = top.enter_context(tc.tile_pool(name="psum", bufs=1, space="PSUM"))

        # ---- output accumulator (bf16) ----
        oacc = top.enter_context(tc.tile_pool(name="oacc", bufs=1))
        o_ac = [oacc.tile([128, NS], bf16, tag=f"o{dt}", name=f"oac{dt}")
                for dt in range(ET)]

        # ---- resident Q features + rotating K-feature / V slots ----
        qres = top.enter_context(tc.tile_pool(name="qres", bufs=1))
        qa8 = qres.tile([128, NSLOT, NS], f8, tag="qa8")
        kpool = top.enter_context(tc.tile_pool(name="kres", bufs=2))
        vpool = top.enter_context(tc.tile_pool(name="vres", bufs=1))
        vres = [vpool.tile([128, 4, D], bf16, tag=f"v{i}", name=f"v{i}")
                for i in range(2)]

        p1 = top.enter_context(tc.tile_pool(name="p1sb", bufs=2))
        w1 = top.enter_context(tc.tile_pool(name="w1", bufs=1))
        pch = top.enter_context(tc.tile_pool(name="pch", bufs=2))
        epool = top.enter_context(tc.tile_pool(name="epool", bufs=8))
        pnorm = top.enter_context(tc.tile_pool(name="pnorm", bufs=1))

        wq = [w1.tile([128, FP8_FROM * 128], f32r, tag=f"wq{d}", name=f"wq{d}")
              for d in range(ET)]
        wk = [w1.tile([128, FP8_FROM * 128], f32r, tag=f"wk{d}", name=f"wk{d}")
              for d in range(ET)]
        wq8 = [w1.tile([128, 2, D], f8, tag=f"wq8{m}", name=f"wq8{m}")
               for m in range(ET // 2)]
        wk8 = [w1.tile([128, 2, D], f8, tag=f"wk8{m}", name=f"wk8{m}")
               for m in range(ET // 2)]
        wv = [w1.tile([128, D], f32r, tag=f"wv{d}", name=f"wv{d}")
              for d in range(ET)]

        def qslot(et, cs, qsb):
            return qa8[:, 2 * et + cs, qsb * 512 : qsb * 512 + 512]

        def load_xblk(col0, between=None):
            br = p1.tile([128, ET * 512], f32r, tag="xbr", name="xbr")
            # two half-loads so the first projection chain starts sooner
            for h in range(2):
                nc.sync.dma_start(
                    br[:, h * 4 * 512 : (h + 1) * 4 * 512]
                    .rearrange("p (d s) -> p d s", d=4),
                    xT[h * 4 * 128 : (h + 1) * 4 * 128, col0 : col0 + 512]
                    .rearrange("(d p) s -> p d s", p=128))
                if h == 0 and between is not None:
                    between()
            x8 = p1.tile([128, ET, 512], f8, tag="x8", name="x8")
            nc.sync.dma_start(
                x8[:],
                XT8[:, col0 : col0 + 512].rearrange("(d p) s -> p d s", p=128))
            return br, x8

        def theta_chain(xb, et, w_tiles, w8_tiles, bias_tiles, r_on_dve=False):
            """Projection + range reduction; returns (f, g) turn tiles for Sin."""
            br, x8 = xb
            ps = psum.tile([128, 512], f32, tag="proj", name="psf", bufs=4)
            if et < FP8_FROM:
                for d in range(ET):
                    nc.tensor.matmul(ps[:], w_tiles[d][:, et * 128 : (et + 1) * 128],
                                     br[:, d * 512 : (d + 1) * 512],
                                     start=(d == 0), stop=(d == ET - 1))
            else:
                for m in range(ET // 2):
                    nc.tensor.matmul(ps[:],
                                     w8_tiles[m][:, :, et * 128 : (et + 1) * 128],
                                     x8[:, 2 * m : 2 * m + 2, :],
                                     start=(m == 0), stop=(m == ET // 2 - 1),
                                     perf_mode=PM.DoubleRow)
            r = pch.tile([128, 512], f32, tag="r", name="r")
            if r_on_dve:
                nc.vector.tensor_scalar(r[:], ps[:], sc2[et][:], bias_tiles[et][:],
                                        Alu.mult, Alu.add)
            else:
                nc.scalar.activation(r[:], ps[:], Act.Identity,
                                     scale=sc2[et][:], bias=bias_tiles[et][:])
            kk = pch.tile([128, 512], f32, tag="kk", name="kk")
            nc.gpsimd.tensor_scalar(kk[:], r[:], MAGIC, MAGIC, Alu.add, Alu.subtract)
            f = pch.tile([128, 512], f32, tag="f", name="f")
            nc.vector.scalar_tensor_tensor(f[:], kk[:], -1.0, r[:],
                                           Alu.mult, Alu.add)
            g = pch.tile([128, 512], f32, tag="kk", name="g")
            nc.vector.add_range_wrap(g[:], f[:], 0.25, 0.5, 1.0)
            return f, g

        ps_rs = psum.tile([2, NS], f32, tag="rs", bufs=1)
        e_group = []
        e8_pairs = []
        kq = []  # deferred (sblk, kres_tile) for score emission

        def kfeature_unit(xb, kr, et):
            f, g = theta_chain(xb, et, wk, wk8, bk2)
            s32 = pch.tile([128, 512], f32, tag="s32", name="s32")
            nc.scalar.activation(s32[:], f[:], Act.Sin, scale=TWOPI)
            nc.gpsimd.tensor_scalar(kr[:, 2 * et + 1, :], s32[:],
                                    nas[et][:], None, Alu.add)
            c32 = pch.tile([128, 512], f32, tag="s32", name="c32")
            nc.scalar.activation(c32[:], g[:], Act.Sin, scale=TWOPI)
            nc.gpsimd.tensor_scalar(kr[:, 2 * et, :], c32[:],
                                    nac[et][:], None, Alu.add)

        def qfeature_unit(xb, qsb, et):
            f, g = theta_chain(xb, et, wq, wq8, bq2, r_on_dve=True)
            nc.scalar.activation(qslot(et, 1, qsb), f[:], Act.Sin, scale=TWOPI)
            nc.scalar.activation(qslot(et, 0, qsb), g[:], Act.Sin, scale=TWOPI)

        def vproj_unit(xb, sblk, ti):
            br = xb[0]
            for dg in range(2):
                psv = psum.tile([128, 512], f32, tag="proj", name="psv",
                                bufs=4)
                for d in range(ET):
                    nc.tensor.matmul(
                        psv[:],
                        br[:, d * 512 + ti * 128 : d * 512 + (ti + 1) * 128],
                        wv[d][:, dg * 512 : dg * 512 + 512],
                        start=(d == 0), stop=(d == ET - 1))
                nc.vector.tensor_copy(
                    vres[sblk % 2][:, ti, dg * 512 : (dg + 1) * 512], psv[:])

        def emit_block_units(xb, kr, sblk):
            """K features (+ Q features for the query blocks) with the V
            projection interleaved so the PE never starves while ACT/DVE/Pool
            chew the feature chains."""
            units = []
            for et in range(ET):
                units.append(lambda et=et: kfeature_unit(xb, kr, et))
                if sblk < n_qsb:
                    units.append(lambda et=et: qfeature_unit(xb, sblk, et))
            vus = [lambda ti=ti: vproj_unit(xb, sblk, ti) for ti in range(4)]
            # pattern: two feature units up front, then alternate
            order = []
            vi = 0
            for i, u in enumerate(units):
                order.append(u)
                if i >= 1 and (i % (len(units) // 4)) == 1 and vi < 4:
                    order.append(vus[vi]); vi += 1
            order.extend(vus[vi:])
            for u in order:
                u()

        def emit_scores(kr, sblk):
            for loc in range(4):
                tt = sblk * 4 + loc
                et_t = epool.tile([128, NS], bf16, tag="e", name="e")
                for ns in range(n_ns):
                    sl = slice(ns * 512, ns * 512 + 512)
                    ps_sim = psum.tile([128, 512], f32, tag="big",
                                       name="ps_sim", bufs=2)
                    for j in range(ET):
                        nc.tensor.matmul(
                            ps_sim[:],
                            kr[:, 2 * j : 2 * j + 2, loc * 128 : (loc + 1) * 128],
                            qa8[:, 2 * j : 2 * j + 2, ns * 512 : ns * 512 + 512],
                            start=(j == 0), stop=(j == ET - 1),
                            perf_mode=PM.DoubleRow)
                    nc.scalar.activation(et_t[:, sl], ps_sim[:], Act.Exp,
                                         scale=INV_SQRT_D)
                e_group.append((tt, et_t))

        def emit_av(tg):
            # rowsums first (exps are long done; avoids pacing PE on ACT)
            for tt, et_t in e_group:
                for ns in range(n_ns):
                    sl = slice(ns * 512, ns * 512 + 512)
                    nc.tensor.matmul(ps_rs[:, sl], ones_col[:], et_t[:, sl],
                                     start=(tt == 0), stop=(tt == n_tt - 1))
            if tg == n_sblk // 2 - 1:
                # rowsum chain closed: 1/rowsum + broadcast overlap final AV
                rec = pnorm.tile([1, NS], bf16, tag="rec")
                with nc.allow_low_precision(
                        reason="1/rowsum broadcast runs bf16; it feeds the "
                               "bf16 bc tile"):
                    nc.vector.reciprocal(rec[:], ps_rs[:1, :])
                bc_t = pnorm.tile([128, NS], bf16, tag="bc")
                for ns in range(n_ns):
                    sl = slice(ns * 512, ns * 512 + 512)
                    ps_bc = psum.tile([128, 512], f32, tag="big", name="ps_bc",
                                      bufs=2)
                    nc.tensor.matmul(ps_bc[:], ones_row[:], rec[:, sl],
                                     start=True, stop=True)
                    nc.vector.tensor_copy(bc_t[:, sl], ps_bc[:])
                bc.append(bc_t)
            for dg in range(2):
                for di in range(4):
                    dt = dg * 4 + di
                    for ns in range(n_ns):
                        sl = slice(ns * 512, ns * 512 + 512)
                        ps_o = psum.tile([128, 512], f32, tag="big", name="ps_o",
                                         bufs=2)
                        for gi in range(8):
                            g_s, ti = gi // 4, gi % 4
                            nc.tensor.matmul(
                                ps_o[:],
                                vres[g_s][:, ti, dt * 128 : (dt + 1) * 128],
                                e_group[gi][1][:, sl],
                                start=(gi == 0), stop=(gi == 7))
                        if tg == 0:
                            nc.vector.tensor_copy(o_ac[dt][:, sl], ps_o[:])
                        else:
                            nc.vector.tensor_tensor(o_ac[dt][:, sl], ps_o[:],
                                                    o_ac[dt][:, sl], Alu.add)
                    if tg == n_sblk // 2 - 1:
                        # final group: normalize + V bias + store right away so
                        # the tail pipelines behind the remaining AV chains
                        on = p1.tile([128, NS], f32, tag="xbr", name="on")
                        nc.vector.tensor_tensor(on[:], o_ac[dt][:], bc[0][:],
                                                Alu.mult)
                        nc.scalar.activation(on[:], on[:], Act.Identity,
                                             bias=bvt[dt][:])
                        nc.sync.dma_start(OT[dt * 128 : (dt + 1) * 128, :], on[:])

        def emit_scores_and_maybe_av(s_i, kr_i):
            emit_scores(kr_i, s_i)
            if len(e_group) == 8:
                emit_av(s_i // 2)
                e_group.clear()

        # ---- main pipeline over key blocks ----
        bc = []
        def load_wk():
            for d in range(ET):
                nc.sync.dma_start(wk[d][:], WkT[d * 128 : (d + 1) * 128, :])

        for sblk in range(n_sblk):
            xb = load_xblk(sblk * 512, between=load_wk if sblk == 0 else None)
            if sblk == 0:
                # remaining weights behind the first x block on the DMA queue
                for d in range(ET):
                    nc.sync.dma_start(wq[d][:], WqT[d * 128 : (d + 1) * 128, :])
                for m in range(ET // 2):
                    nc.sync.dma_start(
                        wk8[m][:], WK8[2 * m * 128 : (2 * m + 2) * 128, :]
                        .rearrange("(j p) e -> p j e", p=128))
                for m in range(ET // 2):
                    nc.sync.dma_start(
                        wq8[m][:], WQ8[2 * m * 128 : (2 * m + 2) * 128, :]
                        .rearrange("(j p) e -> p j e", p=128))
                for d in range(ET):
                    nc.sync.dma_start(wv[d][:], WvT[d * 128 : (d + 1) * 128, :])
            kr = kpool.tile([128, NSLOT, 512], f8, tag="kr", name=f"kr{sblk}")
            kq.append((sblk, kr))
            if sblk == n_qsb:
                # qa8 complete: flush scores (+AV) of the query blocks BEFORE
                # vproj(sblk) (inside emit_block_units) reuses their vres slots
                while kq[0][0] < n_qsb:
                    s_i, kr_i = kq.pop(0)
                    emit_scores_and_maybe_av(s_i, kr_i)
            emit_block_units(xb, kr, sblk)
            if sblk >= n_qsb:
                s_i, kr_i = kq.pop(0)
                emit_scores_and_maybe_av(s_i, kr_i)

    nc.compile()
    return nc


def _host_prep(x, Wq, bq, Wk, bk, Wv, bv, phase_bias):
    wavelengths = np.arange(1, D + 1, dtype=np.float32) * np.float32(2.0 * math.pi / D)
    inv_wl = (np.float32(1.0) / (wavelengths + np.float32(1e-8))).astype(np.float32)
    sc2 = (inv_wl / TWOPI).astype(np.float32).reshape(ET, 128)
    bq2 = ((bq * inv_wl + phase_bias) / TWOPI).astype(np.float32).reshape(ET, 128)
    bk2 = ((bk * inv_wl + phase_bias) / TWOPI).astype(np.float32).reshape(ET, 128)
    # K-feature means from the weights: theta_k ~ N(bk*ivl + pb, |wk_row|^2 ivl^2)
    mu = (bk * inv_wl + phase_bias).astype(np.float64)
    var = (np.sum(Wk.astype(np.float64) ** 2, axis=1) * inv_wl.astype(np.float64) ** 2)
    damp = np.exp(-var / 2.0)
    nac = (-(np.cos(mu) * damp)).astype(np.float32).reshape(ET, 128)
    nas = (-(np.sin(mu) * damp)).astype(np.float32).reshape(ET, 128)
    WqTf = np.ascontiguousarray(Wq.T).astype(np.float32)
    WkTf = np.ascontiguousarray(Wk.T).astype(np.float32)
    WqT = np.ascontiguousarray(WqTf[:, : FP8_FROM * 128])
    WkT = np.ascontiguousarray(WkTf[:, : FP8_FROM * 128])
    WQ8 = WqTf.astype(F8NP)
    WK8 = WkTf.astype(F8NP)
    WvT = np.ascontiguousarray(Wv.T).astype(np.float32)
    xT = [np.ascontiguousarray(x[b].T).astype(np.float32) for b in range(x.shape[0])]
    con = np.stack([sc2, bq2, bk2, bv.reshape(ET, 128).astype(np.float32), nac, nas])
    # [6, ET, 128] -> [128, 6*ET] with column layout (kind, et)
    con = np.ascontiguousarray(con.reshape(6 * ET, 128).T).astype(np.float32)
    return xT, WqT, WkT, WQ8, WK8, WvT, con


def kernel(x, Wq, bq, Wk, bk, Wv, bv, phase_bias, _trace=False):
    from concourse.bass_utils import run_bass_kernel_spmd

    x = np.asarray(x, dtype=np.float32)
    xT, WqT, WkT, WQ8, WK8, WvT, con = _host_prep(
        x, np.asarray(Wq, np.float32), np.asarray(bq, np.float32),
        np.asarray(Wk, np.float32), np.asarray(bk, np.float32),
        np.asarray(Wv, np.float32), np.asarray(bv, np.float32),
        np.asarray(phase_bias, np.float32))

    if "prog" not in _cache:
        _cache["prog"] = _build_program()
    nc = _cache["prog"]

    in_maps = []
    for c in range(NCORES):
        b, qb = c // 4, c % 4
        # rotate the key blocks so this core's query rows come first
        # (scores/rowsum/AV are invariant under key permutation)
        xp = np.concatenate(
            [xT[b][:, qb * QBLK : (qb + 1) * QBLK],
             xT[b][:, : qb * QBLK],
             xT[b][:, (qb + 1) * QBLK :]], axis=1)
        xp = np.ascontiguousarray(xp)
        in_maps.append({
            "xT": xp,
            "XT8": xp.astype(F8NP),
            "WqT": WqT, "WkT": WkT, "WQ8": WQ8, "WK8": WK8, "WvT": WvT,
            "CON": con,
        })
    res = run_bass_kernel_spmd(nc, in_maps, core_ids=list(range(NCORES)),
                               trace=_trace)
    out = np.empty((B, S, D), dtype=np.float32)
    for c in range(NCORES):
        b, qb = c // 4, c % 4
        out[b, qb * QBLK : (qb + 1) * QBLK, :] = res.results[c]["OT"].T
    if _trace:
        kernel.last_exec_time_ns = res.exec_time_ns
        kernel.last_result = res
    return out
